# revision 2
# baseline (speedup 1.0000x reference)
"""MultiHeadSelfAttention2D Trainium2 kernel.

Full inputs -> shard batch (B=8) across 8 NeuronCores (1 image per core) ->
bass/Tile attention kernel per core -> gather.

v2 changes over the 693us baseline (see trace analysis):
  * PV col-tiled 2-way: head pairs at PE array columns 0 and 64 write
    disjoint partition ranges (0-32 / 64-96) of ONE PSUM bank -> the two
    PV matmuls of a pair run concurrently (halves PV wall time, was 311us
    serial).  PSUM has_written init is done by one zeroing outer-product
    matmul per bank (start=True over all 128 partitions); all PV matmuls
    are accumulate-only, so no two start=True matmuls ever share a bank.
  * All 4 QK matmuls of a kblk issued adjacently -> 4-way row-tile
    concurrency (span ~= 1 LDW + 1 MM instead of 2x(LDW+MM)).
  * S^T ring deepened to 3 slots (6 banks) so QK(g) only waits on
    exp(g-3): decouples the ~1.0-1.2us exp ops from the PE period.
  * ot (PV accumulator) shrunk to [128,1024] = 2 banks, released ~1us
    after the last PV by a single ScalarE PSUM->SBUF copy; the whole
    normalize (Z reciprocal bounce + broadcast + multiply) runs out of
    the SBUF copy, deferred into the next q-block's stream so neither
    the PE nor the exp engines ever stall on it (was 8x ~8us PE stalls
    + HAM re-throttle).
  * Projections run in bf16 (fp32 moving operands stream ~3x slower).
  * exp split ScalarE:DVE rebalanced ~36:28 (measured 997 vs 1192+extras
    ns/op).
out = sum_h wo_h.T @ (of_h * (1/Z_h)) + (bo + Wo @ bv)
"""

import numpy as np

EMBED = 128
HEADS = 4
HD = 32
P = 128

_CACHE = {}

# Schraudolph bf16 exp on DVE: i16 = round(S * EXPA + EXPB) bitcast bf16
_EXP_C = 0.0430
EXPA = float(128.0 * 1.4426950408889634)
EXPB = float(128.0 * (127.0 - _EXP_C))

LAG = 8          # pair-groups the PV matmuls trail the QK/exp stream
NSLOT = 3        # S^T ring depth (pair slots, 2 PSUM banks each)

_DRAIN_MAX_WAITS = 1


def _split_multiwait_drains(nc, mybir, bass_rust):
    """Move excess sync-waits onto standalone same-engine EventSemaphore
    instructions (the dispatching sequencer executes them in program order,
    so this is semantically identical)."""
    for fn in nc.m.functions:
        for bb in fn.blocks:
            new = []
            changed = False
            for inst in bb.instructions:
                si = inst.sync_info
                if (si is not None
                        and len(si.on_wait) > _DRAIN_MAX_WAITS):
                    changed = True
                    waits = list(si.on_wait)
                    for j, w in enumerate(waits[_DRAIN_MAX_WAITS:]):
                        es = mybir.InstEventSemaphore(
                            name=f"{inst.name}-wsplit{j}", ins=[], outs=[])
                        es.engine = inst.engine
                        es.sync_info = bass_rust.SyncInfo(
                            on_wait=[w], on_update=[])
                        nc.register_instruction(es)
                        new.append(es)
                    inst.sync_info = bass_rust.SyncInfo(
                        on_wait=waits[:_DRAIN_MAX_WAITS],
                        on_update=list(si.on_update))
                new.append(inst)
            if changed:
                bb.instructions = new


def _exp_on_scalar(G):
    """Engine for the exp of global pair-group G.

    Strict alternation (hf=0 -> ScalarE, hf=1 -> DVE) keeps the
    slot-ring chain exp(G-3) -> QK(G) -> exp(G) bouncing between the
    two engines (no same-engine runs lengthening the chain); every
    16th kblk both slots go to ScalarE to rebalance total load
    (ScalarE exp op is ~11% faster and DVE carries the normalize
    multiplies)."""
    k, hf = G // 2, G % 2
    return hf == 0 or (k % 16) == 15


def _build_nc(n_tokens):
    import bass_rust
    import concourse.bass as bass
    import concourse.tile as tile
    import concourse.mybir as mybir

    f32 = mybir.dt.float32
    f32r = mybir.dt.float32r
    bf16 = mybir.dt.bfloat16
    i16 = mybir.dt.int16
    AF = mybir.ActivationFunctionType
    ADD = mybir.AluOpType.add
    MUL = mybir.AluOpType.mult
    N = n_tokens
    NQB = N // 512          # query blocks of 512
    NKB = N // 128          # key blocks of 128

    nc = bass.Bass("TRN2", target_bir_lowering=False, debug=False)

    x_d = nc.dram_tensor("x", [P, N], bf16, kind="ExternalInput").ap()
    wq_d = nc.dram_tensor("wq_t", [P, P], bf16, kind="ExternalInput").ap()
    wk_d = nc.dram_tensor("wk_t", [P, P], bf16, kind="ExternalInput").ap()
    wv_d = nc.dram_tensor("wv_t", [P, P], bf16, kind="ExternalInput").ap()
    wop_d = [nc.dram_tensor(f"wop_{j}", [P, P], bf16,
                            kind="ExternalInput").ap() for j in range(2)]
    bq_d = nc.dram_tensor("bq", [P, 1], f32, kind="ExternalInput").ap()
    bk_d = nc.dram_tensor("bk", [P, 1], f32, kind="ExternalInput").ap()
    bo_d = nc.dram_tensor("bo", [P, 1], f32, kind="ExternalInput").ap()
    vones_d = nc.dram_tensor("vones", [P, P], bf16, kind="ExternalInput").ap()
    out_d = nc.dram_tensor("out", [P, N], f32, kind="ExternalOutput").ap()
    # Z bounce scratch: layout per q: [Z_h0 | Z_h2 | Z_h1 | Z_h3] (512 ea)
    scr_d = [nc.dram_tensor(f"zscr{i}", [1, 2048], f32, kind="Internal").ap()
             for i in range(NQB)]
    scr2_d = [nc.dram_tensor(f"rscr{i}", [1, 2048], f32, kind="Internal").ap()
              for i in range(NQB)]

    with tile.TileContext(nc) as tc:
        _frees = []

        def ptile(shape, name, dt=None):
            t, f = tc.tile(shape, dt or f32, name=name)
            _frees.append(f)
            return t

        # ---- persistent SBUF tensors ----
        wq = ptile([P, P], "wq", bf16)
        wk = ptile([P, P], "wk", bf16)
        wv = ptile([P, P], "wv", bf16)
        wop = [ptile([P, P], f"wop{j}", bf16) for j in range(2)]
        bq = ptile([P, 1], "bq_t")
        bk = ptile([P, 1], "bk_t")
        bo = ptile([P, 1], "bo_t")
        qf = ptile([P, N], "qf", bf16)
        kf = ptile([P, N], "kf", bf16)
        vaug = ptile([P, NKB * 132], "vaug", bf16)
        # of2[j]: bank-j pairs for all q: head 2j on parts 0-31,
        # head 2j+1 on parts 64-95 (parts 32-63/96-127 garbage, zeroed
        # out by wop's zero rows)
        of2 = [ptile([P, N], f"of2_{j}", bf16) for j in range(2)]
        # zeros; partition-0 rows feed the PSUM-init outer-product matmul
        zpad = ptile([P, 512], "zpad", bf16)

        # DMA priority: what the first projection matmuls need comes first
        nc.sync.dma_start(wq[:], wq_d)
        xb = []
        _xbfrees = []
        for i in range(N // 512):
            tb, xbf = tc.tile([P, 512], bf16, name=f"xb{i}")
            _xbfrees.append(xbf)
            nc.sync.dma_start(tb[:], x_d[:, i * 512:(i + 1) * 512])
            xb.append(tb)
        nc.sync.dma_start(wk[:], wk_d)
        nc.sync.dma_start(wv[:], wv_d)
        nc.sync.dma_start(bq[:], bq_d)
        nc.sync.dma_start(bk[:], bk_d)
        nc.vector.memset(zpad[:], 0.0)

        # vaug: per kblk a 132-wide block of 4x [Vh(32) | 1]
        ones_ap = vaug[:].rearrange(
            "p (j c) -> p j c", c=33)[:, :, 32]
        nc.sync.dma_start(ones_ap, vones_d[:, 0:NKB * HEADS])
        for j in range(2):
            nc.sync.dma_start(wop[j][:], wop_d[j])
        nc.sync.dma_start(bo[:], bo_d)

        # ---- phase 1: projections (bf16) ----
        # engine split: Q biases on DVE, K biases on ScalarE, V copies
        # half-and-half, so no single engine walls the phase.
        with tc.tile_pool(name="pproj", bufs=2, space="PSUM") as pproj:
            # PE warm-up: the first ~3.4us of matmul activity runs at the
            # HAM-throttled 1.2 GHz; burn that window on dummy matmuls
            # while the x DMAs land so phase 1 starts at full clock.
            for w in range(14):
                wt = pproj.tile([P, 512], f32, tag="warm")
                nc.tensor.matmul(wt[:], zpad[0:1, 0:P], zpad[0:1, 0:512],
                                 start=True, stop=True)
            for i in range(NQB):
                ps = pproj.tile([P, 512], f32, tag="ps")
                nc.tensor.matmul(ps[:], wq[:], xb[i][:], start=True, stop=True)
                nc.vector.tensor_scalar(
                    qf[:, i * 512:(i + 1) * 512], ps[:], bq[:], None, ADD)
                ps2 = pproj.tile([P, 512], f32, tag="ps")
                nc.tensor.matmul(ps2[:], wk[:], xb[i][:], start=True, stop=True)
                nc.scalar.activation(
                    kf[:, i * 512:(i + 1) * 512], ps2[:], AF.Identity,
                    bias=bk[:], scale=1.0)
            for k in range(NKB):
                vp = pproj.tile([P, 128], f32, tag="vp")
                xsl = xb[k // 4][:, (k % 4) * 128:(k % 4 + 1) * 128]
                nc.tensor.matmul(vp[:], xsl, wv[:], start=True, stop=True)
                dst = vaug[:, k * 132:(k + 1) * 132].rearrange(
                    "p (h c) -> p h c", h=HEADS)[:, :, 0:32]
                src = vp[:].rearrange("p (h c) -> p h c", h=HEADS)
                if k % 2 == 0:
                    nc.vector.tensor_copy(dst, src)
                else:
                    nc.scalar.activation(dst, src, AF.Copy)

        # ---- phase 2: attention ----
        with tc.tile_pool(name="sring", bufs=1, space="PSUM") as sp, \
             tc.tile_pool(name="opool", bufs=1, space="PSUM") as opl, \
             tc.tile_pool(name="ptpool", bufs=LAG + 3) as ptp, \
             tc.tile_pool(name="ocpool", bufs=2) as ocp, \
             tc.tile_pool(name="rhpool", bufs=2) as rhp, \
             tc.tile_pool(name="nrm", bufs=2) as nrm:
            slots = [sp.tile([P, 1024], f32, name=f"slot{i}")
                     for i in range(NSLOT)]

            # work deferred into the NEXT q-block's instruction stream so
            # neither the PE nor the exp engines ever idle at q boundaries:
            carry = []      # leftover PV pairs (write the PREVIOUS ot)
            rel = []        # ot release: oc copy + Z-row DMAs
            st_recip = []   # reciprocal + broadcast DMA chain
            st_mul = []     # the of = oc * (1/Z) multiplies

            def make_pv(ptt, k, hf, ot):
                def pv():
                    for j in (0, 1):
                        h = 2 * hf + j
                        nc.tensor.matmul(
                            ot[64 * j:64 * j + 33,
                               hf * 512:hf * 512 + 512],
                            vaug[:, k * 132 + 33 * h:k * 132 + 33 * h + 33],
                            ptt[:, j * 512:(j + 1) * 512],
                            start=False, stop=(k == NKB - 1),
                            tile_position=(0, 64 * j),
                            skip_group_check=True)
                return pv

            def make_release(ot, q, fast=False):
                def release():
                    oc = ocp.tile([P, 1024], f32, tag="oc")
                    nc.scalar.activation(oc[:], ot[:], AF.Copy)
                    # Z rows: parts 32 ([Z0|Z2]) and 96 ([Z1|Z3]) -> DRAM
                    scr = scr_d[q]
                    nc.sync.dma_start(scr[0:1, 0:1024], oc[32:33, :])
                    nc.sync.dma_start(scr[0:1, 1024:2048], oc[96:97, :])
                    cmp_ = nrm.tile([P, 16], f32, tag="cmp")
                    nc.sync.dma_start(
                        cmp_[:], scr.rearrange("o (p f) -> (o p) f", p=P))

                    def recip():
                        cmpr = nrm.tile([P, 16], f32, tag="cmpr")
                        nc.vector.reciprocal(cmpr[:], cmp_[:])
                        scr2 = scr2_d[q]
                        nc.sync.dma_start(
                            scr2.rearrange("o (p f) -> (o p) f", p=P),
                            cmpr[:])
                        rh = rhp.tile([P, 1024], f32, tag="rh")
                        # bank j: parts 0-63 <- 1/Z_{2j}, 64-127 <- 1/Z_{2j+1}
                        nc.sync.dma_start(
                            rh[0:64, 0:512],
                            scr2[0:1, 0:512].partition_broadcast(64))
                        nc.sync.dma_start(
                            rh[64:128, 0:512],
                            scr2[0:1, 1024:1536].partition_broadcast(64))
                        nc.sync.dma_start(
                            rh[0:64, 512:1024],
                            scr2[0:1, 512:1024].partition_broadcast(64))
                        nc.sync.dma_start(
                            rh[64:128, 512:1024],
                            scr2[0:1, 1536:2048].partition_broadcast(64))
                        _mk_mul(q, oc, rh, fast)
                    st_recip.append(recip)
                return release

            def _mk_mul(q, oc, rh, fast=False):
                def mul():
                    # GpSimd is otherwise idle; keep the exp engines clear.
                    # Last q-block: DVE (idle by then, and ~2x faster) so
                    # the tail chain is short.
                    eng = nc.vector if fast else nc.gpsimd
                    for j in (0, 1):
                        eng.tensor_mul(
                            of2[j][:, q * 512:(q + 1) * 512],
                            oc[:, j * 512:(j + 1) * 512],
                            rh[:, j * 512:(j + 1) * 512])
                st_mul.append(mul)

            from collections import deque
            pvq = deque()

            for q in range(NQB):
                ot = opl.tile([P, 1024], f32, tag="ot")
                qs = slice(q * 512, (q + 1) * 512)

                for k in range(NKB):
                    G0 = 64 * q + 2 * k
                    # all 4 QK matmuls adjacent -> row-tile overlap
                    for hf in (0, 1):
                        slot = slots[(G0 + hf) % NSLOT]
                        for j in (0, 1):
                            h = 2 * hf + j
                            nc.tensor.matmul(
                                slot[:, j * 512:(j + 1) * 512],
                                kf[32 * h:32 * (h + 1),
                                   k * 128:(k + 1) * 128],
                                qf[32 * h:32 * (h + 1), qs],
                                start=True, stop=True,
                                tile_position=(32 * h, 0))
                    for hf in (0, 1):
                        G = G0 + hf
                        slot = slots[G % NSLOT]
                        ptt = ptp.tile([P, 1024], bf16, tag="pt")
                        if _exp_on_scalar(G):
                            nc.scalar.activation(ptt[:], slot[:], AF.Exp)
                        else:
                            nc.vector.tensor_scalar(
                                ptt[:].bitcast(i16), slot[:],
                                EXPA, EXPB, MUL, ADD)
                        pvq.append(make_pv(ptt, k, hf, ot))
                    # carried PV drain from the previous q-block (k 0-2)
                    for _ in range(4):
                        if carry:
                            carry.pop(0)()
                    if k == 2 and rel:
                        rel.pop(0)()
                    if k == 3:
                        # one start=True matmul per bank zeroes the whole
                        # bank (all 128 partitions) and sets has_written,
                        # so the PV stream is pure accumulate.
                        for j in (0, 1):
                            nc.tensor.matmul(
                                ot[0:P, j * 512:(j + 1) * 512],
                                zpad[0:1, 0:P], zpad[0:1, 0:512],
                                start=True, stop=False,
                                skip_group_check=True)
                    if k == 8 and st_recip:
                        st_recip.pop(0)()
                    if k == 20 and st_mul:
                        st_mul.pop(0)()
                    while len(pvq) > LAG:
                        pvq.popleft()()
                carry.extend(pvq)
                pvq.clear()
                rel.append(make_release(ot, q, fast=(q == NQB - 1)))

            while carry:
                carry.pop(0)()
            while rel:
                rel.pop(0)()
            while st_recip:
                st_recip.pop(0)()
            while st_mul:
                st_mul.pop(0)()

        # ---- phase 3: output projection ----
        with tc.tile_pool(name="pout", bufs=2, space="PSUM") as pop, \
             tc.tile_pool(name="osb", bufs=2) as osbp:
            for i in range(NQB):
                po = pop.tile([P, 512], f32, tag="po")
                for j in range(2):
                    nc.tensor.matmul(po[:], wop[j][:],
                                     of2[j][:, i * 512:(i + 1) * 512],
                                     start=(j == 0), stop=(j == 1))
                ob = osbp.tile([P, 512], f32, tag="ob")
                nc.scalar.activation(ob[:], po[:], AF.Identity,
                                     bias=bo[:], scale=1.0)
                nc.sync.dma_start(out_d[:, i * 512:(i + 1) * 512], ob[:])

        for xbf in reversed(_xbfrees):
            xbf()
        for f in reversed(_frees):
            f()

    _split_multiwait_drains(nc, mybir, bass_rust)
    return nc


def prep_weights(Wq, bq, Wk, bk, Wv, bv, Wo, bo):
    """Host-side weight preprocessing (numpy)."""
    import ml_dtypes
    s = np.float32(1.0 / np.sqrt(HD))
    wq_t = np.ascontiguousarray((s * Wq).T).astype(ml_dtypes.bfloat16)
    bq_s = np.ascontiguousarray((s * bq).reshape(P, 1)).astype(np.float32)
    wk_t = np.ascontiguousarray(Wk.T).astype(ml_dtypes.bfloat16)
    bk_c = np.ascontiguousarray(bk.reshape(P, 1)).astype(np.float32)
    wv_t = np.ascontiguousarray(Wv.T).astype(ml_dtypes.bfloat16)
    bo_f = np.ascontiguousarray((bo + Wo @ bv).reshape(P, 1)).astype(np.float32)
    d = dict(wq_t=wq_t, bq=bq_s, wk_t=wk_t, bk=bk_c, wv_t=wv_t, bo=bo_f,
             vones=np.ones((P, P), ml_dtypes.bfloat16))
    for j in range(2):
        wp = np.zeros((P, P), np.float32)
        wp[0:32, :] = Wo[:, 32 * (2 * j):32 * (2 * j) + 32].T
        wp[64:96, :] = Wo[:, 32 * (2 * j + 1):32 * (2 * j + 1) + 32].T
        d[f"wop_{j}"] = wp.astype(ml_dtypes.bfloat16)
    return d


LAST_RESULTS = None


def kernel(x, Wq, bq, Wk, bk, Wv, bv, Wo, bo):
    global LAST_RESULTS
    import os
    from concourse.bass_utils import run_bass_kernel_spmd

    x = np.asarray(x, np.float32)
    B, C, H, W = x.shape
    N = H * W
    key = ("nc", N)
    if key not in _CACHE:
        _CACHE[key] = _build_nc(N)
    nc = _CACHE[key]

    wmap = prep_weights(np.asarray(Wq, np.float32), np.asarray(bq, np.float32),
                        np.asarray(Wk, np.float32), np.asarray(bk, np.float32),
                        np.asarray(Wv, np.float32), np.asarray(bv, np.float32),
                        np.asarray(Wo, np.float32), np.asarray(bo, np.float32))

    import ml_dtypes
    in_maps = []
    for b in range(B):
        m = dict(wmap)
        m["x"] = np.ascontiguousarray(
            x[b].reshape(C, N).astype(ml_dtypes.bfloat16))
        in_maps.append(m)

    tmpdir = os.environ.get("KERNEL_TMPDIR") or None
    # Rare device-residue corruption shows up as astronomically large
    # outputs: sanity-check and retry; the NEFF is cached so a retry
    # costs seconds.
    for attempt in range(4):
        res = run_bass_kernel_spmd(nc, in_maps, core_ids=list(range(B)),
                                   tmpdir=tmpdir)
        LAST_RESULTS = res
        out = np.stack([res.results[b]["out"] for b in range(B)], axis=0)
        if np.isfinite(out).all() and np.abs(out).max() < 1e3:
            break
    return out.reshape(B, C, H, W).astype(np.float32)
